# revision 69
# baseline (speedup 1.0000x reference)
"""DiT attention (B=2, T=2048, D=1024, H=16, rope on head 0) on 8 trn2 cores.

Sharding: tensor-parallel over heads. Core c owns heads {2c, 2c+1}:
  - QKV projection in fp8e4m3 hi/lo split (x and 32*w each split into
    fp8 + fp8 residual on the host; three DoubleRow passes hi*hi + hi*lo +
    lo*hi accumulate in psum) - same accuracy as bf16 at 3/4 the PE cost.
    Column-sharded (384 features per core), x^T replicated and pre-tiled.
  - Q^T/K^T kept transposed [dims, tokens] bf16; V projected directly in
    natural [tokens, dims] layout (x^T as the stationary operand), with a
    resident ones-column per key tile for the softmax denominator.
  - Attention fully local per (batch, head); scores computed per 128-key tile
    (S^T = K^T' @ Q^T) in bf16, exp evicts PSUM->SBUF bf16 on the ACT engine.
  - AV transposed: pt chunks [keys, 128q] are the stationary operand, vn
    [keys, 65] moving -> av2 psum [128 q, 65] per (qchunk, head) costs 65
    free rows/matmul instead of 512 (the cost model charges output free
    size). Col 64 = softmax denominator via the vn ones-column;
    normalization is folded into the psum->sbuf eviction (one wide
    reciprocal + one broadcast-AP multiply); PE transposes (via identity)
    restore [dims, tokens] for the out-proj. Each av2 psum bank is opened
    once by a bank-wide zeroing matmul and all AV matmuls accumulate with
    start=False (hw: an interleaved start=True wipes other in-flight
    groups' data in the same bank).
  - Out projection row-sharded bf16; per-core partial summed on host.
Scheduling: one software-pipelined stream. Projections split into a qk part
(gates scores) and a V part (drained just before the AV matmuls that need
it); each unit prefetches the next unit's first two score tiles so the ACT
engine (exp, the binding floor) never sees a unit-boundary gap; the next
batch's projections drain during the previous batch's last units.
"""
import sys
sys.path.insert(0, "/opt/trn_rl_repo")
import numpy as np

B, T, D, H, HD = 2, 2048, 1024, 16, 64
NCORES = 8
NTOK = B * T            # 4096
NG = 8                  # 512-token groups (b*4 + tt)
KC = 8                  # contraction chunks of 128 over D
KCP = 4                 # DoubleRow kc pairs
NKT = T // 128          # 16 key tiles per batch
QC = 4                  # 512-query chunks per batch
ROPE_BASE = 10000.0
WSCALE = 32.0           # host pre-scale on w_qkv so the fp8-lo residual
                        # stays in e4m3 normal range; undone via exp scale
                        # (1/WSCALE^2) and w_out/WSCALE

_CACHE = {}
DEBUG_DUMP = False


def _build():
    import concourse.bacc as bacc
    import concourse.mybir as mybir
    import concourse.tile as tile

    F32 = mybir.dt.float32
    BF16 = mybir.dt.bfloat16
    F8 = mybir.dt.float8e4
    EXP = mybir.ActivationFunctionType.Exp
    DR = mybir.MatmulPerfMode.DoubleRow

    nc = bacc.Bacc("TRN2", target_bir_lowering=False, debug=False, num_devices=NCORES)

    # xt8: [128, kc-pair-major] fp8 hi/lo; layout per group g:
    #   col g*4096 + kc*512 + c  (kc pairs (2i,2i+1) adjacent -> DoubleRow)
    xth_d = nc.dram_tensor("xth", [128, NG * KC * 512], F8, kind="ExternalInput")
    xtl_d = nc.dram_tensor("xtl", [128, NG * KC * 512], F8, kind="ExternalInput")
    # wq8: per kc-pair p, per ft (q,k,v): [kc=2p ft-block 128 | kc=2p+1 ...]
    #   col (p*3 + ft)*256 + tt*128 + c
    wqh_d = nc.dram_tensor("wqh", [128, KC * 384], F8, kind="ExternalInput")
    wql_d = nc.dram_tensor("wql", [128, KC * 384], F8, kind="ExternalInput")
    wout = nc.dram_tensor("wout", [128, D], BF16, kind="ExternalInput")
    cosT = nc.dram_tensor("cosT", [64, T], BF16, kind="ExternalInput")
    sinT = nc.dram_tensor("sinT", [64, T], BF16, kind="ExternalInput")
    maskb = nc.dram_tensor("maskb", [128, B * NKT], F32, kind="ExternalInput")
    ident = nc.dram_tensor("ident", [128, 128], BF16, kind="ExternalInput")
    out_d = nc.dram_tensor("out", [128, B * QC * 4096], BF16, kind="ExternalOutput")
    if DEBUG_DUMP:
        qt_d = nc.dram_tensor("qt_dbg", [128, T], BF16, kind="ExternalOutput")
        kt_d = nc.dram_tensor("kt_dbg", [128, T], BF16, kind="ExternalOutput")
        vn_d = nc.dram_tensor("vn_dbg", [128, 2 * NKT * 65], BF16,
                              kind="ExternalOutput")

    with tile.TileContext(nc) as tc:
        with (
            tc.tile_pool(name="consts", bufs=1) as consts,
            tc.tile_pool(name="resid", bufs=1) as resid,
            tc.tile_pool(name="xtp", bufs=1) as xtp,
            tc.tile_pool(name="ptp", bufs=4) as ptp,
            tc.tile_pool(name="rotp", bufs=3) as rotp,
            tc.tile_pool(name="smallp", bufs=3) as smallp,
            tc.tile_pool(name="outst", bufs=2) as outstp,
            tc.tile_pool(name="stgp", bufs=2) as stgp,
            tc.tile_pool(name="ps_sc", bufs=2, space="PSUM") as ps_sc,
            tc.tile_pool(name="ps_av", bufs=1, space="PSUM") as ps_av,
            tc.tile_pool(name="ps_pj", bufs=2, space="PSUM") as ps_pj,
        ):
            # ---- constants + x^T tiles, ordered to shorten the critical
            # startup chain (first scores need wq + xt0 + cos/sin) ----
            ident_sb = consts.tile([128, 128], BF16)
            nc.sync.dma_start(ident_sb[:], ident[:])
            wqh_sb = consts.tile([128, KC * 384], F8)
            nc.sync.dma_start(wqh_sb[:], wqh_d[:])
            xth = [xtp.tile([128, KC * 512], F8, name=f"xth{g}") for g in range(NG)]
            xtl = [xtp.tile([128, KC * 512], F8, name=f"xtl{g}") for g in range(NG)]

            def load_xt(g):
                nc.sync.dma_start(xth[g][:], xth_d[:, g * 4096:(g + 1) * 4096])
                nc.sync.dma_start(xtl[g][:], xtl_d[:, g * 4096:(g + 1) * 4096])

            # startup-critical loads are spread across three DGE queues
            # (SP/DVE/ACT) so they land in ~4us instead of serializing at
            # ~1.3us each on one queue
            nc.scalar.dma_start(xth[0][:, 2048:4096], xth_d[:, 2048:4096])
            nc.sync.dma_start(xth[0][:, 0:2048], xth_d[:, 0:2048])
            mb_sb = consts.tile([128, B * NKT], F32)
            nc.gpsimd.dma_start(mb_sb[:], maskb[:])
            wql_sb = consts.tile([128, KC * 384], F8)
            nc.scalar.dma_start(wql_sb[:], wql_d[:])
            nc.sync.dma_start(xtl[0][:, 0:2048], xtl_d[:, 0:2048])
            nc.scalar.dma_start(xtl[0][:, 2048:4096], xtl_d[:, 2048:4096])
            # only the first 512 cols of cos/sin block the first rope; the
            # rest can land after the next x tile
            cos_sb = consts.tile([64, T], BF16)
            nc.sync.dma_start(cos_sb[:, 0:512], cosT[:, 0:512])
            sin_sb = consts.tile([64, T], BF16)
            nc.sync.dma_start(sin_sb[:, 0:512], sinT[:, 0:512])
            load_xt(1)
            nc.sync.dma_start(cos_sb[:, 512:], cosT[:, 512:])
            nc.sync.dma_start(sin_sb[:, 512:], sinT[:, 512:])
            for g in range(2, NG):
                load_xt(g)
            wout_sb = consts.tile([128, D], BF16)
            nc.sync.dma_start(wout_sb[:], wout[:])

            # ---- resident per-batch tensors ----
            qt_sb = [resid.tile([128, T], BF16, name=f"qt{b}") for b in range(B)]
            kt_sb = [resid.tile([128, T], BF16, name=f"kt{b}") for b in range(B)]
            # V natural layout: per batch [128 keys, (h,kt) blocks of 65]
            # (col 64 of each block stays 1.0 from the initial memset -> the
            # softmax denominator rides along as col 64 of each av2 block)
            vn_sb = [resid.tile([128, 2 * NKT * 65], BF16, name=f"vn{b}") for b in range(B)]
            for b in range(B):
                nc.gpsimd.memset(vn_sb[b][:], 1.0)
            # zero stationary for the av2 bank openers (hw psum quirk: an
            # interleaved start=True matmul wipes other in-flight groups'
            # first contribution in the same bank, so each av2 bank is opened
            # once with a bank-wide zeroing matmul and every AV matmul
            # accumulates with start=False)
            zs_sb = resid.tile([64, 128], BF16, name="zs")
            nc.gpsimd.memset(zs_sb[:], 0.0)

            # PE p-state warmup: ~3us of back-to-back dummy transposes during
            # the initial DMA window so the real pipeline starts at full clock
            # (the cost model halves PE speed for the first 3us after idle)
            wu = ps_pj.tile([128, 128], BF16, name="warmup", tag="pj")
            for _ in range(16):
                nc.tensor.transpose(wu[:], ident_sb[:], ident_sb[:])

            def pull(bgs, want_pe=False, budget=2):
                """Advance background generators; with want_pe, keep going
                until `budget` items that issued PE work (so exp-latency gaps
                in the foreground stream get matmul filler)."""
                steps, got_pe = 0, 0
                while bgs and steps < 10:
                    try:
                        tag = next(bgs[0])
                    except StopIteration:
                        bgs.pop(0)
                        continue
                    steps += 1
                    if not want_pe:
                        return
                    if tag == "pe":
                        got_pe += 1
                        if got_pe >= budget or len(bgs) <= 1:
                            return

            def dr3(ps_out, lhs_of_pass, rhs_of_pass, nj):
                """Three hi/lo DoubleRow passes accumulating into ps_out.
                lhs_of_pass/rhs_of_pass: (pas, kcp) -> AP."""
                for pas in range(3):
                    for kcp in range(KCP):
                        nc.tensor.matmul(
                            ps_out, lhs_of_pass(pas, kcp), rhs_of_pass(pas, kcp),
                            start=(pas == 0 and kcp == 0),
                            stop=(pas == 2 and kcp == KCP - 1),
                            perf_mode=DR,
                        )

            def proj_gen(b, g):
                """QKV projection for (batch b, 512-token group g), fp8 hi/lo
                DoubleRow. K and Q land transposed [dims, tokens] bf16
                (+rope on rows 0:64); yields the "qk" sentinel, then projects
                V into natural [tokens, dims] layout."""
                xh, xl = xth[b * 4 + g], xtl[b * 4 + g]

                def xp(pas, kcp):  # moving x pair [128, 2, 512]
                    src = xl if pas == 2 else xh
                    return src[:, kcp * 1024:(kcp + 1) * 1024].rearrange(
                        "p (t c) -> p t c", t=2)

                sl = slice(g * 512, (g + 1) * 512)
                # group 0: Q first (the startup-critical first scores need
                # the whole 512-query block, so it heads the DVE rope chain).
                # groups 1-3: K first (only K is deadline-critical for the
                # in-loop score gates; Q is not needed until unit (b, g)).
                # K is evicted+roped in halves so the first score tiles can
                # start after half a K block.
                order = ((0, qt_sb[b]), (1, kt_sb[b])) if g == 0 else \
                    ((1, kt_sb[b]), (0, qt_sb[b]))
                for ft, dst in order:
                    ps = ps_pj.tile([128, 512], F32, name=f"pj{b}{g}{ft}", tag="pj")

                    def wp(pas, kcp, ft=ft):  # stationary w pair [128, 2, 128]
                        # pass 0: xh*wh, pass 1: xh*wl, pass 2: xl*wh
                        src = (wqh_sb, wql_sb, wqh_sb)[pas]
                        base = (kcp * 3 + ft) * 256
                        return src[:, base:base + 256].rearrange(
                            "p (t c) -> p t c", t=2)

                    for pas in range(3):
                        for kcp in range(KCP):
                            nc.tensor.matmul(
                                ps[:], wp(pas, kcp), xp(pas, kcp),
                                start=(pas == 0 and kcp == 0),
                                stop=(pas == 2 and kcp == KCP - 1),
                                perf_mode=DR,
                            )
                        if pas < 2:
                            yield "pe"
                    # rotate-half copies run on DVE (cross-partition copies;
                    # validated on hw). K's evict+rope is split in halves so
                    # the first score tiles (which only need the leading key
                    # columns) unblock sooner; Q (needed whole) is one shot.
                    rot = rotp.tile([64, 512], BF16, name=f"rot{b}{g}{ft}", tag="rot")
                    for hv in range(2 if ft == 1 else 1):
                        n = 256 if ft == 1 else 512
                        hs = slice(g * 512 + hv * n, g * 512 + (hv + 1) * n)
                        rs = slice(hv * n, (hv + 1) * n)
                        nc.vector.tensor_copy(dst[:, hs], ps[:, rs])
                        yield "pe"
                        nc.vector.tensor_copy(rot[0:32, rs], dst[32:64, hs])
                        nc.vector.tensor_copy(rot[32:64, rs], dst[0:32, hs])
                        yield
                        nc.vector.tensor_mul(rot[:, rs], rot[:, rs], sin_sb[:, hs])
                        nc.vector.tensor_mul(dst[0:64, hs], dst[0:64, hs],
                                             cos_sb[:, hs])
                        yield
                        nc.vector.tensor_add(dst[0:64, hs], dst[0:64, hs],
                                             rot[:, rs])
                        yield
                yield "qk"
                psv = ps_pj.tile([128, 512], F32, name=f"pv{b}{g}", tag="pj")
                for j in range(4):
                    def xvp(pas, kcp, j=j):  # stationary x pair [128, 2, 128]
                        src = xl if pas == 2 else xh
                        return src[:, kcp * 1024:(kcp + 1) * 1024].rearrange(
                            "p (t c) -> p t c", t=2)[:, :, j * 128:(j + 1) * 128]

                    def wvp(pas, kcp):  # moving w pair [128, 2, 128]
                        src = (wqh_sb, wql_sb, wqh_sb)[pas]
                        base = (kcp * 3 + 2) * 256
                        return src[:, base:base + 256].rearrange(
                            "p (t c) -> p t c", t=2)

                    for pas in range(3):
                        for kcp in range(KCP):
                            nc.tensor.matmul(
                                psv[:, j * 128:(j + 1) * 128],
                                xvp(pas, kcp), wvp(pas, kcp),
                                start=(pas == 0 and kcp == 0),
                                stop=(pas == 2 and kcp == KCP - 1),
                                perf_mode=DR,
                            )
                    yield "pe"
                psv_r = psv[:].rearrange("p (j c) -> p j c", j=4)
                for h in range(2):
                    base = (h * NKT + g * 4) * 65
                    dst = vn_sb[b][:, base:base + 4 * 65].rearrange(
                        "p (j c) -> p j c", j=4)[:, :, 0:64]
                    nc.vector.tensor_copy(dst, psv_r[:, :, h * 64:(h + 1) * 64])
                yield

            # gate state: per (b, g): {"gen": generator|None, "qk": bool}
            gstate = [[{"gen": None, "qk": False} for _ in range(4)] for _ in range(B)]

            def drain_qk(b, g):
                st = gstate[b][g]
                if st["gen"] is None or st["qk"]:
                    return
                for tag in st["gen"]:
                    if tag == "qk":
                        st["qk"] = True
                        return
                st["gen"] = None
                st["qk"] = True

            def drain_full(b, g):
                st = gstate[b][g]
                if st["gen"] is None:
                    return
                for _ in st["gen"]:
                    pass
                st["gen"] = None
                st["qk"] = True

            def step_gate(b, g, n=1):
                """Advance a gate generator a few items (spread its cost)."""
                st = gstate[b][g]
                if st["gen"] is None:
                    return False
                for _ in range(n):
                    try:
                        tag = next(st["gen"])
                        if tag == "qk":
                            st["qk"] = True
                    except StopIteration:
                        st["gen"] = None
                        st["qk"] = True
                        return False
                return True

            sc_t = {}

            def trace_scores(b, qc, kt):
                if (b, qc, kt) in sc_t:
                    return
                q0 = qc * 512
                sc = ps_sc.tile([128, 1024], F32, name=f"sc{b}{qc}{kt}", tag="sc")
                for h in range(2):
                    nc.tensor.matmul(
                        sc[:, h * 512:(h + 1) * 512],
                        kt_sb[b][h * 64:(h + 1) * 64, kt * 128:(kt + 1) * 128],
                        qt_sb[b][h * 64:(h + 1) * 64, q0:q0 + 512],
                        start=True, stop=True,
                    )
                sc_t[(b, qc, kt)] = sc

            def attn_unit(b, qc, bgs, first, nxt, next_is_batch):
                """Attention for (batch b, 512-query chunk qc). `first`: this
                is (b, 0) so qk/V projections gate scores/AV; `nxt`: the next
                unit (its first two score tiles are prefetched at kt 13/14 so
                exp never sees a unit-boundary gap); `next_is_batch`: nxt
                starts the next batch, so its projections drain here."""
                av2 = ps_av.tile([128, 1024], F32, name=f"av{b}{qc}", tag="av")
                for bank in range(2):
                    nc.tensor.matmul(
                        av2[:, bank * 512:(bank + 1) * 512],
                        zs_sb[:], cos_sb[:, 0:512],
                        start=True, stop=False, skip_group_check=True)
                drain_qk(b, 0)
                trace_scores(b, qc, 0)
                for kt in range(NKT):
                    if kt + 1 < NKT:
                        drain_qk(b, (kt + 1) // 4)
                        trace_scores(b, qc, kt + 1)
                    if kt < 2:
                        # PE is in-order: queue filler BEFORE the first AV
                        # matmuls, which stall on the previous chunk's av2
                        # eviction
                        pull(bgs, want_pe=True, budget=1)
                    pt = ptp.tile([128, 1024], BF16, name=f"pt{b}{qc}{kt}", tag="pt")
                    nc.scalar.activation(pt[:], sc_t.pop((b, qc, kt))[:], EXP,
                                         bias=mb_sb[:, b * NKT + kt:b * NKT + kt + 1],
                                         scale=float(HD) ** -0.5 / (WSCALE * WSCALE))
                    drain_full(b, kt // 4)
                    for j in range(4):
                        for h in range(2):
                            blk = (j * 2 + h) * 128
                            nc.tensor.matmul(
                                av2[:, blk:blk + 65],
                                pt[:, h * 512 + j * 128:h * 512 + (j + 1) * 128],
                                vn_sb[b][:, (h * NKT + kt) * 65:(h * NKT + kt + 1) * 65],
                                start=False, stop=(kt == NKT - 1),
                                skip_group_check=True,
                            )
                    if first:
                        # pre-step the next projection gate so its drain is
                        # spread across kts instead of bursting at deadline
                        step_gate(b, min(kt // 4 + 1, 3), n=2)
                    if next_is_batch and kt >= 6:
                        # drain the next batch's first qk projection here so
                        # its scores can be prefetched before this unit ends
                        step_gate(nxt[0], 0, n=2)
                    if nxt is not None:
                        if kt == 12 and next_is_batch:
                            drain_qk(nxt[0], 0)
                        if kt == 13:
                            trace_scores(nxt[0], nxt[1], 0)
                        elif kt == 14:
                            trace_scores(nxt[0], nxt[1], 1)
                    # no background pulls in the last kts: pulled tail work
                    # (po matmuls gated on DVE chains) would sit ahead of the
                    # prefetched next-unit scores in PE program order and
                    # delay the in-order completion count that releases the
                    # next unit's first exp
                    if kt < 12:
                        pull(bgs, want_pe=True, budget=1 if first else 3)

                # Evict av2 right away (frees the banks for the next chunk):
                # per (j, h) block, reciprocal of the denominator column then
                # a normalized psum->sbuf bf16 eviction.
                last = nxt is None
                rec = smallp.tile([128, 8], F32, name=f"rec{b}{qc}", tag="rec")
                avsb = smallp.tile([128, 520], BF16, name=f"avsb{b}{qc}", tag="avsb")
                if last:
                    # final unit: the whole drain chain is exposed, so run it
                    # per query chunk (recip -> normalize -> transpose ->
                    # out-proj -> stage -> DMA), evictions fanned across
                    # DVE+ACT (ACT is idle after the final exp) and out-proj
                    # tiles alternating between the pj and (now free) sc
                    # psum pools
                    out_st = outstp.tile([128, 512], BF16, name=f"os{b}{qc}",
                                         tag="os")
                    g2 = b * QC + qc
                    for j in range(4):
                        a2j = av2[:, j * 256:(j + 1) * 256].rearrange(
                            "p (k c) -> p k c", k=2)
                        rj = rec[:, j * 2:(j + 1) * 2].rearrange(
                            "p (k c) -> p k c", k=2)
                        nc.vector.reciprocal(rj, a2j[:, :, 64:65])
                        nc.vector.tensor_mul(
                            avsb[:, j * 130:(j + 1) * 130].rearrange(
                                "p (k c) -> p k c", k=2),
                            a2j[:, :, 0:65], rj.broadcast_to([128, 2, 65]))
                        tr = ps_pj.tile([128, 128], BF16,
                                        name=f"trl{b}{qc}{j}", tag="pj")
                        for h in range(2):
                            nc.tensor.transpose(
                                tr[h * 64:(h + 1) * 64, :],
                                avsb[:, (j * 2 + h) * 65:(j * 2 + h) * 65 + 64],
                                ident_sb[:])
                        nc.scalar.copy(out_st[:, j * 128:(j + 1) * 128], tr[:])
                        for nt in range(2):
                            pool = ps_sc if nt else ps_pj
                            po = pool.tile([128, 512], F32,
                                           name=f"po{b}{qc}{j}{nt}",
                                           tag="sc" if nt else "pj")
                            nc.tensor.matmul(
                                po[:], out_st[:, j * 128:(j + 1) * 128],
                                wout_sb[:, nt * 512:(nt + 1) * 512],
                                start=True, stop=True,
                            )
                            stgj = stgp.tile([128, 512], BF16,
                                             name=f"stg{b}{qc}{j}{nt}",
                                             tag=f"stgl{j}{nt}", bufs=1)
                            eng = nc.scalar.copy if nt else \
                                nc.vector.tensor_copy
                            eng(stgj[:], po[:])
                            nc.sync.dma_start(
                                out_d[:, g2 * 4096 + (j * 2 + nt) * 512:
                                      g2 * 4096 + (j * 2 + nt + 1) * 512],
                                stgj[:])
                    return iter(())

                nc.vector.reciprocal(
                    rec[:].rearrange("p (k c) -> p k c", k=8),
                    av2[:].rearrange("p (k c) -> p k c", k=8)[:, :, 64:65])
                nc.vector.tensor_mul(
                    avsb[:].rearrange("p (k c) -> p k c", k=8),
                    av2[:].rearrange("p (k c) -> p k c", k=8)[:, :, 0:65],
                    rec[:].rearrange("p (k c) -> p k c", k=8).broadcast_to(
                        [128, 8, 65]))

                def tail():
                    out_st = outstp.tile([128, 512], BF16, name=f"os{b}{qc}", tag="os")
                    g2 = b * QC + qc
                    # all 8 transposes into one 1-bank psum tile, one evict:
                    # pulled into later units this is dependency-free PE work
                    # (avsb complete) so it never stalls the in-order PE
                    tr = ps_pj.tile([128, 512], BF16, name=f"tr{b}{qc}", tag="pj")
                    for j in range(4):
                        for h in range(2):
                            nc.tensor.transpose(
                                tr[h * 64:(h + 1) * 64, j * 128:(j + 1) * 128],
                                avsb[:, (j * 2 + h) * 65:(j * 2 + h) * 65 + 64],
                                ident_sb[:])
                        yield "pe"
                    nc.vector.tensor_copy(out_st[:], tr[:])
                    yield
                    for half in range(2):
                        stg = stgp.tile([128, 2048], BF16, name=f"stg{b}{qc}{half}",
                                        tag=f"stg{half % 2}")
                        for i in range(4):
                            qt, nt = divmod(half * 4 + i, 2)
                            po = ps_pj.tile([128, 512], F32,
                                            name=f"po{b}{qc}{qt}{nt}", tag="pj")
                            nc.tensor.matmul(
                                po[:], out_st[:, qt * 128:(qt + 1) * 128],
                                wout_sb[:, nt * 512:(nt + 1) * 512],
                                start=True, stop=True,
                            )
                            nc.vector.tensor_copy(stg[:, i * 512:(i + 1) * 512],
                                                  po[:])
                            yield "pe"
                        nc.sync.dma_start(
                            out_d[:, g2 * 4096 + half * 2048:g2 * 4096 + (half + 1) * 2048],
                            stg[:])
                        yield

                return tail()

            # ---- schedule: one interleaved stream ----
            for b in range(B):
                for g in range(4):
                    gstate[b][g]["gen"] = proj_gen(b, g)
            units = [(b, qc) for b in range(B) for qc in range(QC)]
            bgs = []

            def gate_bg(nb):
                # next batch's projections as background PE filler
                for g in range(4):
                    while step_gate(nb, g, n=1):
                        yield "pe"

            for idx, (b, qc) in enumerate(units):
                nxt = units[idx + 1] if idx + 1 < len(units) else None
                t = attn_unit(b, qc, bgs, qc == 0, nxt,
                              nxt is not None and nxt[0] != b)
                bgs.append(t)
                if b + 1 < B and qc == 0:
                    bgs.append(gate_bg(b + 1))
            for gen in bgs:
                for _ in gen:
                    pass
            if DEBUG_DUMP:
                nc.sync.dma_start(qt_d[:], qt_sb[0][:])
                nc.sync.dma_start(kt_d[:], kt_sb[0][:])
                nc.sync.dma_start(vn_d[:], vn_sb[0][:])

    nc.compile()
    return nc


def _host_inputs(x, w_qkv, w_out, mask):
    import ml_dtypes
    bf = ml_dtypes.bfloat16
    f8 = ml_dtypes.float8_e4m3fn
    x = np.asarray(x, dtype=np.float32)
    w_qkv = np.asarray(w_qkv, dtype=np.float32)
    w_out = np.asarray(w_out, dtype=np.float32)
    mask = np.asarray(mask)

    # x pre-tiled: xt[p, g*4096 + kc*512 + c] = x[token g*512+c, kc*128+p]
    xt = np.ascontiguousarray(
        x.reshape(NG, 512, KC, 128).transpose(3, 0, 2, 1).reshape(128, NG * KC * 512)
    )
    xth = xt.astype(f8)
    xtl = (xt - xth.astype(np.float32)).astype(f8)

    inv_freq = 1.0 / (ROPE_BASE ** (np.arange(0, HD, 2, dtype=np.float32) / HD))
    t = np.arange(T, dtype=np.float32)
    freqs = np.outer(t, inv_freq)                    # [T, 32]
    cos_r = np.cos(np.concatenate([freqs, freqs], 1)).T.astype(np.float32)  # [64, T]
    sin_half = np.sin(freqs).T.astype(np.float32)    # [32, T]
    sin_r = np.concatenate([-sin_half, sin_half], 0)  # [64, T] signed

    mb = np.zeros((128, B * NKT), dtype=np.float32)
    for b in range(B):
        for kt in range(NKT):
            mb[:, b * NKT + kt] = np.where(mask[b, kt * 128:(kt + 1) * 128], 0.0, -1e30)

    in_maps = []
    for c in range(NCORES):
        cs = slice(c * 128, (c + 1) * 128)
        # per-core w block [1024, 384] scaled, then laid out kc-pair-major:
        # col (kcp*3 + ft)*256 + tt*128 + cc  <- w[kc=2*kcp+tt dims, ft-block]
        wc = np.stack([w_qkv[:, 0:D][:, cs], w_qkv[:, D:2 * D][:, cs],
                       w_qkv[:, 2 * D:3 * D][:, cs]], axis=1) * WSCALE  # [1024,3,128]
        wc = wc.reshape(KCP, 2, 128, 3, 128)          # [kcp, tt, p, ft, cc]
        wc = wc.transpose(2, 0, 3, 1, 4).reshape(128, KC * 384)
        wch = np.ascontiguousarray(wc).astype(f8)
        wcl = np.ascontiguousarray(wc - wch.astype(np.float32)).astype(f8)
        if c == 0:
            cosc, sinc = cos_r, sin_r
        else:
            cosc = np.ones_like(cos_r)
            sinc = np.zeros_like(sin_r)
        in_maps.append({
            "xth": xth,
            "xtl": xtl,
            "wqh": wch,
            "wql": wcl,
            "wout": np.ascontiguousarray(w_out[cs, :] / WSCALE).astype(bf),
            "cosT": cosc.astype(bf),
            "sinT": sinc.astype(bf),
            "maskb": mb,
            "ident": np.eye(128, dtype=bf),
        })
    return in_maps


def kernel(x, w_qkv, w_out, mask):
    if "nc" not in _CACHE:
        _CACHE["nc"] = _build()
    nc = _CACHE["nc"]
    in_maps = _host_inputs(x, w_qkv, w_out, mask)

    from concourse.bass_utils import run_bass_kernel_spmd
    res = run_bass_kernel_spmd(nc, in_maps, core_ids=list(range(NCORES)))
    _CACHE["last_results"] = res

    total = np.zeros((NTOK, D), dtype=np.float32)
    for c in range(NCORES):
        part = np.asarray(res.results[c]["out"]).astype(np.float32)
        # out[p, g2*4096 + qt*1024 + nt*512 + f] -> token g2*512+qt*128+p
        total += part.reshape(128, NG, 4, 2, 512).transpose(1, 2, 0, 3, 4).reshape(NTOK, D)
    return total.reshape(B, T, D)
